# revision 3
# baseline (speedup 1.0000x reference)
import math

import jax
import jax.numpy as jnp
import numpy as np

EPS = 1e-9
B, Q, L, NC = 128, 2048, 64, 8
ZI = -4.0 * math.pi * 1e-9  # constant imaginary part of (q/2)^2 - 4*pi*sld_comp


def _abeles_real(q, thickness, roughness, sld):
    # q: (b,Q) f32; thickness, roughness: (b,L); sld: (b,L+1) — all real f32.
    # Complex math done manually in f32 pairs so the neuron compiler never
    # sees complex64.
    amb = sld[:, 0:1]
    c_n = 4.0 * math.pi * (sld - amb) * 1e-6  # (b, L+1)

    q2 = (q * 0.5) ** 2  # (b, Q)
    zr = q2[:, :, None] - c_n[:, None, :]  # (b, Q, L+1)

    # stable complex sqrt of (zr + i*ZI), ZI < 0
    h = jnp.sqrt(zr * zr + ZI * ZI)
    t = jnp.sqrt(0.5 * (h + jnp.abs(zr)))
    w = (0.5 * -ZI) / t
    pos = zr >= 0.0
    kr = jnp.where(pos, t, w)
    ki = jnp.where(pos, -w, -t)

    kcr, kci = kr[..., :-1], ki[..., :-1]  # (b,Q,L)
    knr, kni = kr[..., 1:], ki[..., 1:]

    t_in = thickness[:, None, :]  # (b,1,L)
    rr2 = -2.0 * (roughness * roughness)[:, None, :]  # (b,1,L)

    # exp(i t k) and exp(-i t k)
    tkr = t_in * kcr
    tki = t_in * kci
    eb = jnp.exp(-tki)
    emb = jnp.exp(tki)
    cb = jnp.cos(tkr)
    sb = jnp.sin(tkr)
    ebr, ebi = eb * cb, eb * sb
    embr, embi = emb * cb, -emb * sb

    # fresnel r_n = (kc - kn)/(kc + kn + EPS) * exp(-2 kc kn rough^2)
    nr, ni = kcr - knr, kci - kni
    dr, di = kcr + knr + EPS, kci + kni
    inv = 1.0 / (dr * dr + di * di)
    ratr = (nr * dr + ni * di) * inv
    rati = (ni * dr - nr * di) * inv

    kkr = kcr * knr - kci * kni
    kki = kcr * kni + kci * knr
    wr = rr2 * kkr
    wi = rr2 * kki
    ew = jnp.exp(wr)
    ewr, ewi = ew * jnp.cos(wi), ew * jnp.sin(wi)

    rnr = ratr * ewr - rati * ewi
    rni = ratr * ewi + rati * ewr

    # matrix entries (real/imag planes), each (b,Q,L)
    m00r, m00i = ebr, ebi
    m01r = rnr * ebr - rni * ebi
    m01i = rnr * ebi + rni * ebr
    m10r = rnr * embr - rni * embi
    m10i = rnr * embi + rni * embr
    m11r, m11i = embr, embi

    # tree-reduce the 2x2 complex matrix product over the layer axis:
    # 6 unrolled levels of big batched elementwise ops instead of a 63-step
    # sequential scan.
    def cmul(ar_, ai_, br_, bi_):
        return ar_ * br_ - ai_ * bi_, ar_ * bi_ + ai_ * br_

    p = [m00r, m00i, m01r, m01i, m10r, m10i, m11r, m11i]
    while p[0].shape[-1] > 1:
        a00r, a00i = p[0][..., 0::2], p[1][..., 0::2]
        a01r, a01i = p[2][..., 0::2], p[3][..., 0::2]
        a10r, a10i = p[4][..., 0::2], p[5][..., 0::2]
        a11r, a11i = p[6][..., 0::2], p[7][..., 0::2]
        b00r, b00i = p[0][..., 1::2], p[1][..., 1::2]
        b01r, b01i = p[2][..., 1::2], p[3][..., 1::2]
        b10r, b10i = p[4][..., 1::2], p[5][..., 1::2]
        b11r, b11i = p[6][..., 1::2], p[7][..., 1::2]

        t1r, t1i = cmul(a00r, a00i, b00r, b00i)
        t2r, t2i = cmul(a01r, a01i, b10r, b10i)
        n00r, n00i = t1r + t2r, t1i + t2i
        t1r, t1i = cmul(a00r, a00i, b01r, b01i)
        t2r, t2i = cmul(a01r, a01i, b11r, b11i)
        n01r, n01i = t1r + t2r, t1i + t2i
        t1r, t1i = cmul(a10r, a10i, b00r, b00i)
        t2r, t2i = cmul(a11r, a11i, b10r, b10i)
        n10r, n10i = t1r + t2r, t1i + t2i
        t1r, t1i = cmul(a10r, a10i, b01r, b01i)
        t2r, t2i = cmul(a11r, a11i, b11r, b11i)
        n11r, n11i = t1r + t2r, t1i + t2i
        p = [n00r, n00i, n01r, n01i, n10r, n10i, n11r, n11i]

    ar, ai = p[0][..., 0], p[1][..., 0]
    cr, ci = p[4][..., 0], p[5][..., 0]
    ar = ar + EPS
    inv2 = 1.0 / (ar * ar + ai * ai)
    qr = (cr * ar + ci * ai) * inv2
    qi = (ci * ar - cr * ai) * inv2
    return qr * qr + qi * qi


_pmapped = jax.pmap(_abeles_real)


def kernel(q, thickness, roughness, sld):
    qs = q.reshape(NC, B // NC, Q)
    ts = thickness.reshape(NC, B // NC, L)
    rs = roughness.reshape(NC, B // NC, L)
    ss = sld.reshape(NC, B // NC, L + 1)
    out = _pmapped(qs, ts, rs, ss)
    return np.asarray(jax.device_get(out)).reshape(B, Q).astype(np.float32)


# revision 6
# speedup vs baseline: 1.3846x; 1.3846x over previous
import math

import jax
import jax.numpy as jnp
import numpy as np

EPS = 1e-9
B, Q, L, NC = 128, 2048, 64, 8
ZI = -4.0 * math.pi * 1e-9  # constant imaginary part of (q/2)^2 - 4*pi*sld_comp


def _abeles_real(q, thickness, roughness, sld):
    # q: (b,Q) f32; thickness, roughness: (b,L); sld: (b,L+1) — all real f32.
    # Complex math done manually in f32 pairs so the neuron compiler never
    # sees complex64.
    amb = sld[:, 0:1]
    c_n = 4.0 * math.pi * (sld - amb) * 1e-6  # (b, L+1)

    q2 = (q * 0.5) ** 2  # (b, Q)
    zr = q2[:, :, None] - c_n[:, None, :]  # (b, Q, L+1)

    # stable complex sqrt of (zr + i*ZI), ZI < 0
    h = jnp.sqrt(zr * zr + ZI * ZI)
    t = jnp.sqrt(0.5 * (h + jnp.abs(zr)))
    w = (0.5 * -ZI) / t
    pos = zr >= 0.0
    kr = jnp.where(pos, t, w)
    ki = jnp.where(pos, -w, -t)

    kcr, kci = kr[..., :-1], ki[..., :-1]  # (b,Q,L)
    knr, kni = kr[..., 1:], ki[..., 1:]

    t_in = thickness[:, None, :]  # (b,1,L)
    rr2 = -2.0 * (roughness * roughness)[:, None, :]  # (b,1,L)

    # exp(i t k) and exp(-i t k)
    tkr = t_in * kcr
    tki = t_in * kci
    eb = jnp.exp(-tki)
    emb = jnp.exp(tki)
    cb = jnp.cos(tkr)
    sb = jnp.sin(tkr)
    ebr, ebi = eb * cb, eb * sb
    embr, embi = emb * cb, -emb * sb

    # fresnel r_n = (kc - kn)/(kc + kn + EPS) * exp(-2 kc kn rough^2)
    nr, ni = kcr - knr, kci - kni
    dr, di = kcr + knr + EPS, kci + kni
    inv = 1.0 / (dr * dr + di * di)
    ratr = (nr * dr + ni * di) * inv
    rati = (ni * dr - nr * di) * inv

    kkr = kcr * knr - kci * kni
    kki = kcr * kni + kci * knr
    wr = rr2 * kkr
    wi = rr2 * kki
    ew = jnp.exp(wr)
    ewr, ewi = ew * jnp.cos(wi), ew * jnp.sin(wi)

    rnr = ratr * ewr - rati * ewi
    rni = ratr * ewi + rati * ewr

    # matrix entries (real/imag planes), each (b,Q,L)
    m00r, m00i = ebr, ebi
    m01r = rnr * ebr - rni * ebi
    m01i = rnr * ebi + rni * ebr
    m10r = rnr * embr - rni * embi
    m10i = rnr * embi + rni * embr
    m11r, m11i = embr, embi

    planes = (m00r, m00i, m01r, m01i, m10r, m10i, m11r, m11i)
    init = tuple(p[..., 0] for p in planes)
    ms = tuple(jnp.moveaxis(p[..., 1:], -1, 0) for p in planes)

    def step(carry, m):
        ar, ai, br, bi, cr, ci, dr_, di_ = carry
        er, ei, fr, fi, gr, gi, hr, hi = m
        nar = ar * er - ai * ei + br * gr - bi * gi
        nai = ar * ei + ai * er + br * gi + bi * gr
        nbr = ar * fr - ai * fi + br * hr - bi * hi
        nbi = ar * fi + ai * fr + br * hi + bi * hr
        ncr = cr * er - ci * ei + dr_ * gr - di_ * gi
        nci = cr * ei + ci * er + dr_ * gi + di_ * gr
        ndr = cr * fr - ci * fi + dr_ * hr - di_ * hi
        ndi = cr * fi + ci * fr + dr_ * hi + di_ * hr
        return (nar, nai, nbr, nbi, ncr, nci, ndr, ndi), None

    (ar, ai, _, _, cr, ci, _, _), _ = jax.lax.scan(step, init, ms)
    ar = ar + EPS
    inv2 = 1.0 / (ar * ar + ai * ai)
    qr = (cr * ar + ci * ai) * inv2
    qi = (ci * ar - cr * ai) * inv2
    return qr * qr + qi * qi


_pmapped = jax.pmap(_abeles_real)


def kernel(q, thickness, roughness, sld):
    qs = q.reshape(NC, B // NC, Q)
    ts = thickness.reshape(NC, B // NC, L)
    rs = roughness.reshape(NC, B // NC, L)
    ss = sld.reshape(NC, B // NC, L + 1)
    out = _pmapped(qs, ts, rs, ss)
    return np.asarray(jax.device_get(out)).reshape(B, Q).astype(np.float32)
